# revision 21
# baseline (speedup 1.0000x reference)
"""Banded-causal complex attention on 8 Trainium2 NeuronCores.

Strategy: data-parallel over batch (B=8 -> 1 batch per core). Per core:
  - everything on the input path is bf16 (host-converted): halves HBM
    traffic (6.7MB -> 3.3MB), halves LDWEIGHTS (fast weight load), and
    runs the PE at 1 col/cycle for every matmul shape (no f32r
    moving-dim>=256 constraint).
  - Q is packed [Wqr|Wqi]*scale^2*temp, K is packed [Wkr|-Wki]: the
    complex score real part (qr.kr - qi.ki)*scale*temp is ONE K=128
    matmul per key block.
  - V is projected x-as-weights (lhsT = xT block, rhs = Wv chunk), which
    lands V directly in [key, d] layout -- no PE transposes. bv is folded
    to the end: out = o/l + bv exactly, since softmax rows sum to 1.
  - scores are computed transposed: sT_kb[key c, query r] covers the two
    query blocks (kb, kb+1) that attend key block kb, one N=256 matmul.
  - band+causal masking is one DVE multiply by a precomputed [c,2,r]
    0/1 mask over both halves of exp(sT).
  - softmax skips the max-subtraction (scores are O(1); masked entries
    are exactly zero) and row-sums ride along as a ones column in V.
  - the PE is warmed with ~3.5us of junk matmuls sized to end when the
    first x piece lands, so the HAM clock-gate lifts to 2.4GHz before
    real work starts and never re-throttles.
  - per-piece DMA is split across both HWDGE rings (sync+scalar) with
    pos-table slices interleaved in consumption order.
  - output is normalized (r = 1/l on DVE, fused *r + bv) and DMA'd out
    per 2 query blocks, overlapped under remaining compute.
"""

import numpy as np

B, S, D, KD = 8, 2048, 512, 64
P = 128              # partition size / query block
NB = S // P          # 16 query/key blocks
DCH = D // P         # 4 contraction chunks
NCH = 4              # column pieces
NSL = S // NCH       # 512 columns per piece
NCORES = 8
NJUNK = 32           # PE warmup matmuls (256 cols each, ~213ns cold)

_CACHE = {}
TRACE_KWARGS = {}    # test harness may set e.g. {"trace": True, "tmpdir": ...}


def _build_nc():
    import concourse.bacc as bacc
    import concourse.tile as tile
    import concourse.mybir as mybir
    from concourse.bass import ts

    f32 = mybir.dt.float32
    bf16 = mybir.dt.bfloat16
    nc = bacc.Bacc(None)

    xtr = nc.declare_dram_parameter("xtr", [NCH, P, DCH, NSL], bf16, isOutput=False)
    wq = nc.declare_dram_parameter("wq", [P, DCH, P], bf16, isOutput=False)
    wk = nc.declare_dram_parameter("wk", [P, DCH, P], bf16, isOutput=False)
    wv = nc.declare_dram_parameter("wv", [P, DCH, KD], bf16, isOutput=False)
    pq = nc.declare_dram_parameter("pq", [P, S], bf16, isOutput=False)
    pk = nc.declare_dram_parameter("pk", [P, S], bf16, isOutput=False)
    msk = nc.declare_dram_parameter("msk", [P, 2, P], bf16, isOutput=False)
    bvrow = nc.declare_dram_parameter("bvrow", [1, KD], bf16, isOutput=False)
    out = nc.declare_dram_parameter("out", [S, KD], f32, isOutput=True)

    with tile.TileContext(nc) as tc:
        with (
            tc.tile_pool(name="consts", bufs=1) as consts,
            tc.tile_pool(name="persist", bufs=1) as persist,
            tc.tile_pool(name="work", bufs=5) as work,
            tc.tile_pool(name="rwork", bufs=3) as rwork,
            tc.tile_pool(name="ps_proj", bufs=2, space="PSUM") as ps_proj,
            tc.tile_pool(name="ps_v", bufs=2, space="PSUM") as ps_v,
            tc.tile_pool(name="ps_s", bufs=2, space="PSUM") as ps_s,
            tc.tile_pool(name="ps_o", bufs=2, space="PSUM") as ps_o,
        ):
            xT_sb = persist.tile([P, DCH, S], bf16)
            pq_sb = persist.tile([P, S], bf16)
            pk_sb = persist.tile([P, S], bf16)
            wq_sb = consts.tile([P, DCH, P], bf16)
            wk_sb = consts.tile([P, DCH, P], bf16)
            wv_sb = consts.tile([P, DCH, KD], bf16)
            msk_sb = consts.tile([P, 2, P], bf16)
            bvrow_sb = consts.tile([1, KD], bf16)

            # junk tile fill is GpSimd's FIRST instruction so PE warmup can
            # begin the moment the engines wake
            junk = consts.tile([P, 2 * P], bf16)
            nc.gpsimd.memset(junk, 0.0)

            # ---- input DMA. dma_start descriptor generation occupies the
            # issuing engine's sequencer for ~600ns (HWDGE) / ~1us (SWDGE),
            # so bulk traffic is kept off the ACT engine (which must be free
            # for exp/copies by ~11us): SP carries wq + x halves + pq,
            # GpSimd (SWDGE, otherwise idle) carries pk + x2b/x3b + msk,
            # ACT only issues 4 small/early transfers.
            HS = S // 2
            nc.gpsimd.dma_start(out=bvrow_sb, in_=bvrow[:])
            nc.gpsimd.dma_start(out=msk_sb, in_=msk[:])
            nc.gpsimd.dma_start(
                out=xT_sb[:, 2:4, 2 * NSL : 3 * NSL], in_=xtr[2][:, 2:4, :]
            )
            nc.gpsimd.dma_start(out=pk_sb[:, HS:S], in_=pk[:, HS:S])
            nc.gpsimd.dma_start(
                out=xT_sb[:, 2:4, 3 * NSL : 4 * NSL], in_=xtr[3][:, 2:4, :]
            )

            nc.sync.dma_start(out=wq_sb, in_=wq[:])
            nc.sync.dma_start(out=xT_sb[:, 0:2, 0:NSL], in_=xtr[0][:, 0:2, :])
            nc.sync.dma_start(out=pq_sb[:, 0:HS], in_=pq[:, 0:HS])
            nc.sync.dma_start(
                out=xT_sb[:, 0:2, NSL : 2 * NSL], in_=xtr[1][:, 0:2, :]
            )
            nc.sync.dma_start(
                out=xT_sb[:, 0:2, 2 * NSL : 3 * NSL], in_=xtr[2][:, 0:2, :]
            )
            nc.sync.dma_start(out=pq_sb[:, HS:S], in_=pq[:, HS:S])
            nc.sync.dma_start(
                out=xT_sb[:, 0:2, 3 * NSL : 4 * NSL], in_=xtr[3][:, 0:2, :]
            )

            nc.scalar.dma_start(out=wk_sb, in_=wk[:])
            nc.scalar.dma_start(out=wv_sb, in_=wv[:])
            nc.scalar.dma_start(
                out=xT_sb[:, 2:4, 0:NSL], in_=xtr[0][:, 2:4, :]
            )
            nc.scalar.dma_start(out=pk_sb[:, 0:HS], in_=pk[:, 0:HS])
            nc.scalar.dma_start(
                out=xT_sb[:, 2:4, NSL : 2 * NSL], in_=xtr[1][:, 2:4, :]
            )

            # warm the ACT exp table off the critical path (after the ACT
            # dma_starts so the table load doesn't block their issue)
            dummy = consts.tile([P, 2], f32)
            nc.vector.memset(dummy, 0.0)
            nc.scalar.activation(
                out=dummy, in_=dummy, func=mybir.ActivationFunctionType.Exp
            )

            # all-ones single-partition row for the bv outer-product matmul
            ones1 = consts.tile([1, P], bf16)
            nc.vector.memset(ones1, 1.0)

            # PE warmup: junk matmuls on the zeroed tile while piece-0 DMA
            # lands. Sized to lift the HAM clock gate (~3.4us busy) right
            # as real work becomes ready.
            for _ in range(NJUNK):
                ps_dum = ps_proj.tile([P, NSL], f32, tag="ps", name="ps_dum")
                nc.tensor.matmul(
                    ps_dum[:, 0 : 2 * P], junk[:, 0:P], junk[:], start=True, stop=True
                )

            # qT padded by one block so every sT matmul is a uniform N=256
            qT_sb = persist.tile([P, S + P], bf16)
            kT_sb = persist.tile([P, S], bf16)
            nc.vector.memset(qT_sb[:, S : S + P], 0.0)

            # v_aug[key, block, 0:64] = v; col 64 = 1.0 (rowsum); col 65 pad
            v_aug = persist.tile([P, NB, KD + 2], bf16)
            nc.vector.memset(v_aug[:, :, KD : KD + 2], 1.0)

            out_sb = persist.tile([P, NB, KD], f32)
            out_r = out.rearrange("(q r) k -> r q k", r=P)

            def proj_qk(n):
                sl = slice(n * NSL, (n + 1) * NSL)
                for grp in range(2):  # 0=q, 1=k
                    w_g = (wq_sb, wk_sb)[grp]
                    ps = ps_proj.tile([P, NSL], f32, tag="ps", name="ps")
                    for c in range(DCH):
                        nc.tensor.matmul(
                            ps,
                            w_g[:, c, :],
                            xT_sb[:, c, sl],
                            start=(c == 0),
                            stop=(c == DCH - 1),
                        )
                    if grp == 0:
                        nc.vector.tensor_add(qT_sb[:, sl], ps, pq_sb[:, sl])
                    else:
                        nc.vector.tensor_add(kT_sb[:, sl], ps, pk_sb[:, sl])

            def proj_v(t):
                # x-as-weights: out[s, d] = sum_c xT[c, s]^T wv[c, d], plus
                # a rank-1 ones^T @ bv_row accumulate for the bias (exact:
                # softmax rows sum to 1, so bv folds through attention)
                ps = ps_v.tile([P, KD], f32, tag="v", name="psv")
                for c in range(DCH):
                    nc.tensor.matmul(
                        ps,
                        xT_sb[:, c, ts(t, P)],
                        wv_sb[:, c, :],
                        start=(c == 0),
                        stop=False,
                    )
                nc.tensor.matmul(
                    ps, ones1[:, :], bvrow_sb[:, :], start=False, stop=True
                )
                nc.vector.tensor_copy(v_aug[:, t, 0:KD], ps)

            p_tiles = {}

            def score_block(kb):
                # sT_kb[c, r]: keys of block kb vs queries of blocks kb,kb+1
                s_ps = ps_s.tile([P, 2 * P], f32, tag="s", name="s_ps")
                nc.tensor.matmul(
                    s_ps,
                    kT_sb[:, ts(kb, P)],
                    qT_sb[:, kb * P : kb * P + 2 * P],
                    start=True, stop=True,
                )
                p_sb = work.tile([P, 2, P], bf16, tag="p_sb")
                nc.scalar.activation(
                    out=p_sb, in_=s_ps.rearrange("c (h r) -> c h r", h=2),
                    func=mybir.ActivationFunctionType.Exp,
                )
                # band+causal: half 0 keeps c <= r (diag block qb=kb),
                # half 1 keeps c >= r (off-diag block qb=kb+1). Early blocks
                # mask on DVE; later ones on GpSimd (free after its DMAs).
                eng = nc.vector if kb < 4 else nc.gpsimd
                eng.tensor_mul(p_sb, p_sb, msk_sb)
                p_tiles[kb] = p_sb

            def attend(qb):
                o_ps = ps_o.tile([P, KD + 2], f32, tag="o", name="o_ps")
                halves = [(p_tiles[qb], 0, qb)]
                if qb > 0:
                    halves.insert(0, (p_tiles[qb - 1], 1, qb - 1))
                for i, (pt, h, kb2) in enumerate(halves):
                    nc.tensor.matmul(
                        o_ps,
                        pt[:, h, :],
                        v_aug[:, kb2, :],
                        start=(i == 0),
                        stop=(i == len(halves) - 1),
                    )
                if qb > 1:
                    p_tiles.pop(qb - 2, None)
                r_blk = rwork.tile([P, 1], f32, tag="r_blk")
                nc.vector.reciprocal(r_blk, o_ps[:, KD : KD + 1])
                nc.scalar.activation(
                    out=out_sb[:, qb, :], in_=o_ps[:, 0:KD],
                    func=mybir.ActivationFunctionType.Copy, scale=r_blk,
                )
                if qb % 2 == 1:
                    nc.sync.dma_start(
                        out=out_r[:, qb - 1 : qb + 1, :],
                        in_=out_sb[:, qb - 1 : qb + 1, :],
                    )

            # ---- software-pipelined schedule over the 4 column pieces
            scored = 0
            attended = 0
            for n in range(NCH):
                proj_qk(n)
                for t in range(4 * n, 4 * (n + 1)):
                    proj_v(t)
                target = min(4 * n + 2, NB - 1) if n < NCH - 1 else NB - 1
                while scored <= target:
                    score_block(scored)
                    scored += 1
                    if scored - attended > 2:
                        attend(attended)
                        attended += 1
            while attended < NB:
                attend(attended)
                attended += 1

    nc.finalize()
    return nc


def _prep_core_inputs(inputs):
    import ml_dtypes

    bf16 = ml_dtypes.bfloat16
    g = lambda k: np.asarray(inputs[k], dtype=np.float32)
    x = g("x")
    scale = 1.0 / np.sqrt(np.float32(KD))
    temp = float(np.asarray(inputs["temperature"]).reshape(-1)[0])
    alpha = scale * temp  # folded (softmax temp) * (score scale)

    wq = np.concatenate([g("Wqr"), g("Wqi")], axis=1) * (scale * alpha)
    pq = np.concatenate(
        [
            g("pos_qr") * alpha + g("bqr") * (scale * alpha),
            g("pos_qi") * alpha + g("bqi") * (scale * alpha),
        ],
        axis=1,
    ).T  # [128, S]
    wk = np.concatenate([g("Wkr"), -g("Wki")], axis=1)
    pk = np.concatenate(
        [g("pos_kr") + g("bkr"), -(g("pos_ki") + g("bki"))], axis=1
    ).T
    wv = g("Wv")
    bvrow = g("bv").reshape(1, KD)

    cc, rr = np.meshgrid(np.arange(P), np.arange(P), indexing="ij")
    msk = np.stack([(cc <= rr), (cc >= rr)], axis=1)  # [c, 2, r]

    pe_pack = lambda w: np.ascontiguousarray(
        w.reshape(DCH, P, w.shape[1]).transpose(1, 0, 2)
    ).astype(bf16)
    shared = {
        "wq": pe_pack(wq),
        "wk": pe_pack(wk),
        "wv": pe_pack(wv),
        "pq": np.ascontiguousarray(pq).astype(bf16),
        "pk": np.ascontiguousarray(pk).astype(bf16),
        "msk": np.ascontiguousarray(msk).astype(bf16),
        "bvrow": np.ascontiguousarray(bvrow).astype(bf16),
    }
    in_maps = []
    for b in range(NCORES):
        m = dict(shared)
        # xtr[n, p, c, j] = x[b].T[c*128+p, n*512+j]
        xT_b = np.ascontiguousarray(x[b].T)
        m["xtr"] = np.ascontiguousarray(
            xT_b.reshape(DCH, P, NCH, NSL).transpose(2, 1, 0, 3)
        ).astype(bf16)
        in_maps.append(m)
    return in_maps


def kernel(**inputs):
    from concourse.bass_utils import run_bass_kernel_spmd

    nc = _CACHE.get("nc")
    if nc is None:
        nc = _CACHE["nc"] = _build_nc()
    in_maps = _prep_core_inputs(inputs)
    res = run_bass_kernel_spmd(
        nc, in_maps, core_ids=list(range(NCORES)), **TRACE_KWARGS
    )
    _CACHE["last_result"] = res
    return np.stack([res.results[b]["out"] for b in range(NCORES)], axis=0)


# revision 23
# speedup vs baseline: 1.0921x; 1.0921x over previous
"""Banded-causal complex attention on 8 Trainium2 NeuronCores.

Strategy: data-parallel over batch (B=8 -> 1 batch per core). Per core:
  - everything on the input path is bf16 (host-converted): halves HBM
    traffic (6.7MB -> 3.3MB), halves LDWEIGHTS (fast weight load), and
    runs the PE at 1 col/cycle for every matmul shape (no f32r
    moving-dim>=256 constraint).
  - Q is packed [Wqr|Wqi]*scale^2*temp, K is packed [Wkr|-Wki]: the
    complex score real part (qr.kr - qi.ki)*scale*temp is ONE K=128
    matmul per key block.
  - V is projected x-as-weights (lhsT = xT block, rhs = Wv chunk), which
    lands V directly in [key, d] layout -- no PE transposes. bv is folded
    to the end: out = o/l + bv exactly, since softmax rows sum to 1.
  - scores are computed transposed: sT_kb[key c, query r] covers the two
    query blocks (kb, kb+1) that attend key block kb, one N=256 matmul.
  - band+causal masking is one DVE multiply by a precomputed [c,2,r]
    0/1 mask over both halves of exp(sT).
  - softmax skips the max-subtraction (scores are O(1); masked entries
    are exactly zero) and row-sums ride along as a ones column in V.
  - the PE is warmed with ~3.5us of junk matmuls sized to end when the
    first x piece lands, so the HAM clock-gate lifts to 2.4GHz before
    real work starts and never re-throttles.
  - per-piece DMA is split across both HWDGE rings (sync+scalar) with
    pos-table slices interleaved in consumption order.
  - output is normalized (r = 1/l on DVE, fused *r + bv) and DMA'd out
    per 2 query blocks, overlapped under remaining compute.
"""

import numpy as np

B, S, D, KD = 8, 2048, 512, 64
P = 128              # partition size / query block
NB = S // P          # 16 query/key blocks
DCH = D // P         # 4 contraction chunks
NCH = 4              # column pieces
NSL = S // NCH       # 512 columns per piece
NCORES = 8
NJUNK = 30           # PE warmup matmuls (256 cols each, ~213ns cold)

_CACHE = {}
TRACE_KWARGS = {}    # test harness may set e.g. {"trace": True, "tmpdir": ...}


def _build_nc():
    import concourse.bacc as bacc
    import concourse.tile as tile
    import concourse.mybir as mybir
    from concourse.bass import ts

    f32 = mybir.dt.float32
    bf16 = mybir.dt.bfloat16
    nc = bacc.Bacc(None)

    xtr = nc.declare_dram_parameter("xtr", [NCH, P, DCH, NSL], bf16, isOutput=False)
    wq = nc.declare_dram_parameter("wq", [P, DCH, P], bf16, isOutput=False)
    wk = nc.declare_dram_parameter("wk", [P, DCH, P], bf16, isOutput=False)
    wv = nc.declare_dram_parameter("wv", [P, DCH, KD], bf16, isOutput=False)
    pq = nc.declare_dram_parameter("pq", [P, S], bf16, isOutput=False)
    pk = nc.declare_dram_parameter("pk", [P, S], bf16, isOutput=False)
    msk = nc.declare_dram_parameter("msk", [P, 2, P], bf16, isOutput=False)
    bvrow = nc.declare_dram_parameter("bvrow", [1, KD], bf16, isOutput=False)
    out = nc.declare_dram_parameter("out", [S, KD], f32, isOutput=True)

    with tile.TileContext(nc) as tc:
        with (
            tc.tile_pool(name="consts", bufs=1) as consts,
            tc.tile_pool(name="persist", bufs=1) as persist,
            tc.tile_pool(name="work", bufs=5) as work,
            tc.tile_pool(name="rwork", bufs=3) as rwork,
            tc.tile_pool(name="ps_proj", bufs=2, space="PSUM") as ps_proj,
            tc.tile_pool(name="ps_v", bufs=2, space="PSUM") as ps_v,
            tc.tile_pool(name="ps_s", bufs=2, space="PSUM") as ps_s,
            tc.tile_pool(name="ps_o", bufs=2, space="PSUM") as ps_o,
        ):
            xT_sb = persist.tile([P, DCH, S], bf16)
            pq_sb = persist.tile([P, S], bf16)
            pk_sb = persist.tile([P, S], bf16)
            wq_sb = consts.tile([P, DCH, P], bf16)
            wk_sb = consts.tile([P, DCH, P], bf16)
            wv_sb = consts.tile([P, DCH, KD], bf16)
            msk_sb = consts.tile([P, 2, P], bf16)
            bvrow_sb = consts.tile([1, KD], bf16)

            # junk tile fill is GpSimd's FIRST instruction so PE warmup can
            # begin the moment the engines wake
            junk = consts.tile([P, 2 * P], bf16)
            nc.gpsimd.memset(junk, 0.0)

            # ---- input DMA. dma_start descriptor generation occupies the
            # issuing engine's sequencer for ~600ns (HWDGE) / ~1us (SWDGE),
            # so bulk traffic is kept off the ACT engine (which must be free
            # for exp/copies by ~11us): SP carries wq + x halves + pq,
            # GpSimd (SWDGE, otherwise idle) carries pk + x2b/x3b + msk,
            # ACT only issues 4 small/early transfers.
            # GpSimd (SWDGE, ~1us Q7 descriptor time each): msk + pk
            # quarters + late x b-halves
            nc.gpsimd.dma_start(out=bvrow_sb, in_=bvrow[:])
            nc.gpsimd.dma_start(out=msk_sb, in_=msk[:])
            nc.gpsimd.dma_start(out=pk_sb[:, 0:NSL], in_=pk[:, 0:NSL])
            nc.gpsimd.dma_start(
                out=xT_sb[:, 2:4, 2 * NSL : 3 * NSL], in_=xtr[2][:, 2:4, :]
            )
            nc.gpsimd.dma_start(
                out=pk_sb[:, NSL : 2 * NSL], in_=pk[:, NSL : 2 * NSL]
            )
            nc.gpsimd.dma_start(
                out=xT_sb[:, 2:4, 3 * NSL : 4 * NSL], in_=xtr[3][:, 2:4, :]
            )
            nc.gpsimd.dma_start(
                out=pk_sb[:, 2 * NSL : 3 * NSL], in_=pk[:, 2 * NSL : 3 * NSL]
            )
            nc.gpsimd.dma_start(
                out=pk_sb[:, 3 * NSL : 4 * NSL], in_=pk[:, 3 * NSL : 4 * NSL]
            )

            # SP: wq + all x a-halves + pq quarters
            nc.sync.dma_start(out=wq_sb, in_=wq[:])
            nc.sync.dma_start(out=xT_sb[:, 0:2, 0:NSL], in_=xtr[0][:, 0:2, :])
            nc.sync.dma_start(
                out=xT_sb[:, 0:2, NSL : 2 * NSL], in_=xtr[1][:, 0:2, :]
            )
            nc.sync.dma_start(out=pq_sb[:, 0:NSL], in_=pq[:, 0:NSL])
            nc.sync.dma_start(
                out=xT_sb[:, 0:2, 2 * NSL : 3 * NSL], in_=xtr[2][:, 0:2, :]
            )
            nc.sync.dma_start(
                out=pq_sb[:, NSL : 2 * NSL], in_=pq[:, NSL : 2 * NSL]
            )
            nc.sync.dma_start(
                out=xT_sb[:, 0:2, 3 * NSL : 4 * NSL], in_=xtr[3][:, 0:2, :]
            )
            nc.sync.dma_start(
                out=pq_sb[:, 2 * NSL : 3 * NSL], in_=pq[:, 2 * NSL : 3 * NSL]
            )
            nc.sync.dma_start(
                out=pq_sb[:, 3 * NSL : 4 * NSL], in_=pq[:, 3 * NSL : 4 * NSL]
            )

            # ACT: only 4 small/early descriptors; the engine must be free
            # for exp/copies by ~11us
            nc.scalar.dma_start(out=wk_sb, in_=wk[:])
            nc.scalar.dma_start(out=wv_sb, in_=wv[:])
            nc.scalar.dma_start(
                out=xT_sb[:, 2:4, 0:NSL], in_=xtr[0][:, 2:4, :]
            )
            nc.scalar.dma_start(
                out=xT_sb[:, 2:4, NSL : 2 * NSL], in_=xtr[1][:, 2:4, :]
            )

            # warm the ACT exp table off the critical path (after the ACT
            # dma_starts so the table load doesn't block their issue)
            dummy = consts.tile([P, 2], f32)
            nc.vector.memset(dummy, 0.0)
            nc.scalar.activation(
                out=dummy, in_=dummy, func=mybir.ActivationFunctionType.Exp
            )

            # all-ones single-partition row for the bv outer-product matmul
            ones1 = consts.tile([1, P], bf16)
            nc.vector.memset(ones1, 1.0)

            # PE warmup: junk matmuls on the zeroed tile while piece-0 DMA
            # lands. Sized to lift the HAM clock gate (~3.4us busy) right
            # as real work becomes ready.
            for _ in range(NJUNK):
                ps_dum = ps_proj.tile([P, NSL], f32, tag="ps", name="ps_dum")
                nc.tensor.matmul(
                    ps_dum[:, 0 : 2 * P], junk[:, 0:P], junk[:], start=True, stop=True
                )

            # qT padded by one block so every sT matmul is a uniform N=256
            qT_sb = persist.tile([P, S + P], bf16)
            kT_sb = persist.tile([P, S], bf16)
            nc.vector.memset(qT_sb[:, S : S + P], 0.0)

            # v_aug[key, block, 0:64] = v; col 64 = 1.0 (rowsum); col 65 pad
            v_aug = persist.tile([P, NB, KD + 2], bf16)
            nc.vector.memset(v_aug[:, :, KD : KD + 2], 1.0)

            out_sb = persist.tile([P, NB, KD], f32)
            out_r = out.rearrange("(q r) k -> r q k", r=P)

            def proj_qk(n):
                sl = slice(n * NSL, (n + 1) * NSL)
                for grp in range(2):  # 0=q, 1=k
                    w_g = (wq_sb, wk_sb)[grp]
                    ps = ps_proj.tile([P, NSL], f32, tag="ps", name="ps")
                    for c in range(DCH):
                        nc.tensor.matmul(
                            ps,
                            w_g[:, c, :],
                            xT_sb[:, c, sl],
                            start=(c == 0),
                            stop=(c == DCH - 1),
                        )
                    if grp == 0:
                        nc.vector.tensor_add(qT_sb[:, sl], ps, pq_sb[:, sl])
                    else:
                        nc.vector.tensor_add(kT_sb[:, sl], ps, pk_sb[:, sl])

            def proj_v(t):
                # x-as-weights: out[s, d] = sum_c xT[c, s]^T wv[c, d], plus
                # a rank-1 ones^T @ bv_row accumulate for the bias (exact:
                # softmax rows sum to 1, so bv folds through attention)
                ps = ps_v.tile([P, KD], f32, tag="v", name="psv")
                for c in range(DCH):
                    nc.tensor.matmul(
                        ps,
                        xT_sb[:, c, ts(t, P)],
                        wv_sb[:, c, :],
                        start=(c == 0),
                        stop=False,
                    )
                nc.tensor.matmul(
                    ps, ones1[:, :], bvrow_sb[:, :], start=False, stop=True
                )
                nc.vector.tensor_copy(v_aug[:, t, 0:KD], ps)

            p_tiles = {}

            def score_block(kb):
                # sT_kb[c, r]: keys of block kb vs queries of blocks kb,kb+1
                s_ps = ps_s.tile([P, 2 * P], f32, tag="s", name="s_ps")
                nc.tensor.matmul(
                    s_ps,
                    kT_sb[:, ts(kb, P)],
                    qT_sb[:, kb * P : kb * P + 2 * P],
                    start=True, stop=True,
                )
                p_sb = work.tile([P, 2, P], bf16, tag="p_sb")
                nc.scalar.activation(
                    out=p_sb, in_=s_ps.rearrange("c (h r) -> c h r", h=2),
                    func=mybir.ActivationFunctionType.Exp,
                )
                # band+causal: half 0 keeps c <= r (diag block qb=kb),
                # half 1 keeps c >= r (off-diag block qb=kb+1). Early blocks
                # mask on DVE; later ones on GpSimd (free after its DMAs).
                eng = nc.vector if kb < 4 else nc.gpsimd
                eng.tensor_mul(p_sb, p_sb, msk_sb)
                p_tiles[kb] = p_sb

            def attend(qb):
                o_ps = ps_o.tile([P, KD + 2], f32, tag="o", name="o_ps")
                halves = [(p_tiles[qb], 0, qb)]
                if qb > 0:
                    halves.insert(0, (p_tiles[qb - 1], 1, qb - 1))
                for i, (pt, h, kb2) in enumerate(halves):
                    nc.tensor.matmul(
                        o_ps,
                        pt[:, h, :],
                        v_aug[:, kb2, :],
                        start=(i == 0),
                        stop=(i == len(halves) - 1),
                    )
                if qb > 1:
                    p_tiles.pop(qb - 2, None)
                r_blk = rwork.tile([P, 1], f32, tag="r_blk")
                nc.vector.reciprocal(r_blk, o_ps[:, KD : KD + 1])
                nc.scalar.activation(
                    out=out_sb[:, qb, :], in_=o_ps[:, 0:KD],
                    func=mybir.ActivationFunctionType.Copy, scale=r_blk,
                )
                if qb % 2 == 1:
                    nc.sync.dma_start(
                        out=out_r[:, qb - 1 : qb + 1, :],
                        in_=out_sb[:, qb - 1 : qb + 1, :],
                    )

            # ---- software-pipelined schedule over the 4 column pieces
            scored = 0
            attended = 0
            for n in range(NCH):
                proj_qk(n)
                for t in range(4 * n, 4 * (n + 1)):
                    proj_v(t)
                target = min(4 * n + 2, NB - 1) if n < NCH - 1 else NB - 1
                while scored <= target:
                    score_block(scored)
                    scored += 1
                    if scored - attended > 2:
                        attend(attended)
                        attended += 1
            while attended < NB:
                attend(attended)
                attended += 1

    nc.finalize()
    return nc


def _prep_core_inputs(inputs):
    import ml_dtypes

    bf16 = ml_dtypes.bfloat16
    g = lambda k: np.asarray(inputs[k], dtype=np.float32)
    x = g("x")
    scale = 1.0 / np.sqrt(np.float32(KD))
    temp = float(np.asarray(inputs["temperature"]).reshape(-1)[0])
    alpha = scale * temp  # folded (softmax temp) * (score scale)

    wq = np.concatenate([g("Wqr"), g("Wqi")], axis=1) * (scale * alpha)
    pq = np.concatenate(
        [
            g("pos_qr") * alpha + g("bqr") * (scale * alpha),
            g("pos_qi") * alpha + g("bqi") * (scale * alpha),
        ],
        axis=1,
    ).T  # [128, S]
    wk = np.concatenate([g("Wkr"), -g("Wki")], axis=1)
    pk = np.concatenate(
        [g("pos_kr") + g("bkr"), -(g("pos_ki") + g("bki"))], axis=1
    ).T
    wv = g("Wv")
    bvrow = g("bv").reshape(1, KD)

    cc, rr = np.meshgrid(np.arange(P), np.arange(P), indexing="ij")
    msk = np.stack([(cc <= rr), (cc >= rr)], axis=1)  # [c, 2, r]

    pe_pack = lambda w: np.ascontiguousarray(
        w.reshape(DCH, P, w.shape[1]).transpose(1, 0, 2)
    ).astype(bf16)
    shared = {
        "wq": pe_pack(wq),
        "wk": pe_pack(wk),
        "wv": pe_pack(wv),
        "pq": np.ascontiguousarray(pq).astype(bf16),
        "pk": np.ascontiguousarray(pk).astype(bf16),
        "msk": np.ascontiguousarray(msk).astype(bf16),
        "bvrow": np.ascontiguousarray(bvrow).astype(bf16),
    }
    in_maps = []
    for b in range(NCORES):
        m = dict(shared)
        # xtr[n, p, c, j] = x[b].T[c*128+p, n*512+j]
        xT_b = np.ascontiguousarray(x[b].T)
        m["xtr"] = np.ascontiguousarray(
            xT_b.reshape(DCH, P, NCH, NSL).transpose(2, 1, 0, 3)
        ).astype(bf16)
        in_maps.append(m)
    return in_maps


def kernel(**inputs):
    from concourse.bass_utils import run_bass_kernel_spmd

    nc = _CACHE.get("nc")
    if nc is None:
        nc = _CACHE["nc"] = _build_nc()
    in_maps = _prep_core_inputs(inputs)
    res = run_bass_kernel_spmd(
        nc, in_maps, core_ids=list(range(NCORES)), **TRACE_KWARGS
    )
    _CACHE["last_result"] = res
    return np.stack([res.results[b]["out"] for b in range(NCORES)], axis=0)
